# revision 23
# baseline (speedup 1.0000x reference)
"""Trainium2 Bass kernel for an RNN-T style JointNet.

Reference computation (per batch element b):
    enc = enc_out @ W_enc.T + b_enc          # (T, J)
    dec = dec_out @ W_dec.T + b_dec          # (U, J)
    h   = tanh(enc[:,None,:] + dec[None,:,:])  # (T, U, J)
    logits = h @ W_fc.T + b_fc               # (T, U, V)
    out = log_softmax(logits, axis=-1)

Sharding: data-parallel over batch — 8 batch elements, one per NeuronCore.
Device layout: features-on-partitions ("transposed") so the vocab axis of the
logits lands on the free dimension, where the vector/scalar engines can reduce.

v2 changes vs the bf16 baseline (449 us):
  - FC matmul runs in fp8e4 with perf_mode=DoubleRow: h is written by the
    tanh ACT directly as fp8, W_fc is host-quantized fp8 scaled by 2^12
    (dodges e4m3 subnormals; max |W_fc|*4096 = 128 < 240). Each (tile, jc
    pair) is one K=256 DoubleRow matmul — half the matmul count, ~1.44x
    tensor-engine throughput. The 2^12 scale is undone by the free affine
    scale on the Exp and by the fused output op.
  - bf16 everywhere on the DVE: enc_lin/dec_lin/comb are bf16, so the join
    broadcast-add runs in the DVE 4x perf mode.
  - psmain has 4 PSUM buffers (projection pools are scoped out first), so
    the PE never stalls on log-softmax draining.
  - Output is written bf16 (halves the output DMA); host upcasts.
  - fc bias b_fc (scaled 2^12, bf16) still enters via rank-1 ones x b_fc
    accumulating matmuls.
  - The final log_softmax subtract is fused (2^-12 scale + neg_lse add) and
    split between DVE (tensor_scalar) and ACT (Identity w/ bias+scale) to
    balance engine load.
"""

import numpy as np
import ml_dtypes

import concourse.bass as bass
import concourse.mybir as mybir
from concourse import bacc
from concourse.tile import TileContext
from concourse.bass_utils import run_bass_kernel_spmd

BF16 = ml_dtypes.bfloat16
FP8 = ml_dtypes.float8_e4m3

# Problem dims (hardcoded, matches the grading harness inputs)
B, T, U, D, J, V = 8, 200, 50, 512, 1024, 1024
PT = 128          # partition tile (rows per fc matmul tile)
DC = D // 128     # 4 contraction chunks for the projections
JC = J // 128     # 8 contraction chunks for the fc matmul
JP = JC // 2      # 4 DoubleRow k-tile pairs
NV2 = V // 2      # 512: one PSUM bank of fp32
UB = 10           # u values per generation block
NBLK = U // UB    # 5 blocks
ROWS = UB * T     # 2000 rows per block
NT = (ROWS + PT - 1) // PT  # 16 tiles per block (last has 80 rows)

WSCALE = 4096.0   # 2^12: fp8 weight scale (keeps e4m3 out of subnormals)
INV_WSCALE = 1.0 / WSCALE

# log2(1+t) ~= C0 + C1 t + C2 t^2 + C3 t^3 + C4 t^4  (max err 2.1e-4, t in [0,1])
C0, C1, C2, C3, C4 = (
    0.000204257, 1.436097109, -0.669512499, 0.312211590, -0.079149584)
LN2 = 0.6931471805599453

_CACHE = {}


OCT = 8           # log-softmax tiles batched per lse computation


def _neg_log_oct(nc, pool, sums):
    """neg_lse = -ln(sums) for a (128, OCT) fp32 SBUF tile, on the DVE.

    s = 2^e * m with m in [1,2): ln(s) = ln2 * ((e+127) + log2(m) - 127).
    """
    i32, f32 = mybir.dt.int32, mybir.dt.float32
    Alu = mybir.AluOpType
    xi = sums.bitcast(i32)
    e_i = pool.tile([128, OCT], i32, tag="lt_ei")
    nc.vector.tensor_scalar(e_i, xi, 23, None, Alu.logical_shift_right)
    e_f = pool.tile([128, OCT], f32, tag="lt_ef")
    nc.vector.tensor_copy(e_f, e_i)  # int32 -> fp32 value conversion
    m_i = pool.tile([128, OCT], i32, tag="lt_mi")
    nc.vector.tensor_scalar(
        m_i, xi, 0x007FFFFF, 0x3F800000, Alu.bitwise_and, Alu.bitwise_or)
    t = pool.tile([128, OCT], f32, tag="lt_t")
    nc.vector.tensor_scalar(t, m_i.bitcast(f32), 1.0, None, Alu.subtract)
    p = pool.tile([128, OCT], f32, tag="lt_p")
    nc.vector.tensor_scalar(p, t, C4, C3, Alu.mult, Alu.add)
    nc.vector.tensor_mul(p, p, t)
    nc.vector.tensor_scalar(p, p, C2, None, Alu.add)
    nc.vector.tensor_mul(p, p, t)
    nc.vector.tensor_scalar(p, p, C1, None, Alu.add)
    nc.vector.tensor_mul(p, p, t)          # p = P(t) - C0
    nc.vector.tensor_add(p, p, e_f)        # p += (e + 127)
    nl = pool.tile([128, OCT], f32, tag="lt_nl")
    nc.vector.tensor_scalar(nl, p, (C0 - 127.0), -LN2, Alu.add, Alu.mult)
    return nl


def build_bass():
    f32, bf16 = mybir.dt.float32, mybir.dt.bfloat16
    fp8 = mybir.dt.float8e4
    AF = mybir.ActivationFunctionType
    Alu = mybir.AluOpType

    # Bacc (not plain Bass): its compile pipeline legalizes multi-sem waits
    # (1 HW wait slot per instruction) and inserts ACT table loads.
    nc = bacc.Bacc(trn_type="TRN2")
    encT = nc.dram_tensor("enct", [D, T], bf16, kind="ExternalInput")
    decT = nc.dram_tensor("dect", [D, U], bf16, kind="ExternalInput")
    wencT = nc.dram_tensor("wenct", [D, J], bf16, kind="ExternalInput")
    wdecT = nc.dram_tensor("wdect", [D, J], bf16, kind="ExternalInput")
    wfcT = nc.dram_tensor("wfct", [J, V], fp8, kind="ExternalInput")
    bjoint = nc.dram_tensor("bjoint", [128, JC], f32, kind="ExternalInput")
    bfc = nc.dram_tensor("bfc", [1, V], bf16, kind="ExternalInput")
    bfcb = nc.dram_tensor("bfcb", [128, V], bf16, kind="ExternalInput")
    out = nc.dram_tensor("out", [T * U, V], bf16, kind="ExternalOutput")

    with TileContext(nc) as tc:
        with (
            tc.tile_pool(name="const", bufs=1) as const_pool,
            tc.tile_pool(name="comb", bufs=4) as comb_pool,
            tc.tile_pool(name="hbuf", bufs=2) as hbuf_pool,
            tc.tile_pool(name="small", bufs=4) as small_pool,
            tc.tile_pool(name="es", bufs=3) as es_pool,
            # ob0 buffers live for a whole block (16 tiles) awaiting the lse
            tc.tile_pool(name="ob0", bufs=18) as ob0_pool,
            tc.tile_pool(name="ob", bufs=8) as ob_pool,
        ):
            # ---- load constants/weights -------------------------------------
            # enc-path tensors first: the projections only need these.
            encT_sb = const_pool.tile([128, DC, T], bf16)
            nc.sync.dma_start(
                out=encT_sb, in_=encT.rearrange("(c p) t -> p c t", p=128))
            wenc_sb = const_pool.tile([128, DC, J], bf16)
            wenc_r = wencT.rearrange("(c p) j -> p c j", p=128)
            for jc in range(JC):
                nc.sync.dma_start(out=wenc_sb[:, :, jc * 128:(jc + 1) * 128],
                                  in_=wenc_r[:, :, jc * 128:(jc + 1) * 128])
            decT_sb = const_pool.tile([128, DC, U], bf16)
            nc.sync.dma_start(
                out=decT_sb, in_=decT.rearrange("(c p) u -> p c u", p=128))
            wdec_sb = const_pool.tile([128, DC, J], bf16)
            wdec_r = wdecT.rearrange("(c p) j -> p c j", p=128)
            for jc in range(JC):
                nc.sync.dma_start(out=wdec_sb[:, :, jc * 128:(jc + 1) * 128],
                                  in_=wdec_r[:, :, jc * 128:(jc + 1) * 128])
            bjoint_sb = const_pool.tile([128, JC], f32)
            nc.sync.dma_start(out=bjoint_sb, in_=bjoint[:, :])
            wfc_sb = const_pool.tile([128, JC, V], fp8)
            wfc_r = wfcT.rearrange("(c p) v -> p c v", p=128)
            for jc in range(JC):
                nc.sync.dma_start(out=wfc_sb[:, jc, :], in_=wfc_r[:, jc, :])
            bfc_sb = const_pool.tile([1, V], bf16)
            nc.sync.dma_start(out=bfc_sb, in_=bfc[:, :])
            # unscaled b_fc broadcast across partitions, for the DVE passA
            bfcb_sb = const_pool.tile([128, V], bf16)
            nc.sync.dma_start(out=bfcb_sb, in_=bfcb[:, :])
            ones_sb = const_pool.tile([1, 128], bf16)
            nc.vector.memset(ones_sb, 1.0)

            # ---- enc/dec projections (feature-on-partition outputs) ---------
            # bf16 outputs: feeds the bf16 4x-mode DVE broadcast-add.
            enc_lin = const_pool.tile([128, JC, T], bf16)
            # f32: tensor_scalar requires an fp32 scalar operand
            dec_lin = const_pool.tile([128, JC, U], f32)
            with (
                # separate 1-buf pools so each projection's first matmul
                # starts on a fresh PSUM slot: matmul instructions have only
                # 2 HW sync-wait slots and the first dec matmul already waits
                # on 2 DMA queues. Scoped: released before psmain opens.
                tc.tile_pool(name="psproj", bufs=1, space="PSUM") as psp,
                tc.tile_pool(name="psdec", bufs=1, space="PSUM") as psd,
            ):
                # interleaved so (enc_lin[jc], dec_lin[jc]) pairs complete
                # early -- block 0's comb/tanh chase right behind
                for jc in range(JC):
                    pe = psp.tile([128, T], f32, tag="proj")
                    for dc in range(DC):
                        nc.tensor.matmul(
                            pe, wenc_sb[:, dc, jc * 128:(jc + 1) * 128],
                            encT_sb[:, dc, :], start=(dc == 0),
                            stop=(dc == DC - 1))
                    nc.scalar.copy(enc_lin[:, jc, :], pe)
                    pd = psd.tile([128, U], f32, tag="dproj")
                    for dc in range(DC):
                        nc.tensor.matmul(
                            pd, wdec_sb[:, dc, jc * 128:(jc + 1) * 128],
                            decT_sb[:, dc, :], start=(dc == 0),
                            stop=(dc == DC - 1))
                    # both biases folded in here: dec_lin += (b_enc + b_dec)
                    nc.scalar.activation(
                        dec_lin[:, jc, :], pd, AF.Identity,
                        bias=bjoint_sb[:, jc:jc + 1], scale=1.0)

            with tc.tile_pool(name="psmain", bufs=4, space="PSUM") as psmain:
                # ---- main loop over u-blocks --------------------------------
                for blk in range(NBLK):
                    h = hbuf_pool.tile([128, JC, ROWS], fp8, tag="h")
                    for jc in range(JC):
                        comb = comb_pool.tile([128, ROWS], bf16, tag="comb")
                        for ul in range(UB):
                            u = blk * UB + ul
                            nc.vector.tensor_scalar(
                                comb[:, ul * T:(ul + 1) * T],
                                enc_lin[:, jc, :],
                                dec_lin[:, jc, u:u + 1], None, Alu.add)
                        nc.scalar.activation(h[:, jc, :], comb, AF.Tanh)

                    oct_ob0 = [None] * OCT
                    oct_m = [0] * OCT
                    oct_r0 = [0] * OCT
                    sums = None
                    for k in range(NT):
                        m = PT if k < NT - 1 else ROWS - PT * (NT - 1)
                        j = k % OCT
                        if j == 0:
                            sums = small_pool.tile(
                                [128, OCT], mybir.dt.float32, tag="sums")
                            nc.vector.memset(sums, 1.0)
                        # Tiles whose passA drains on ACT (engine balance):
                        # ACT can't add the [1,V] bias vector, so these keep
                        # rank-1 bias matmuls. DVE tiles get b_fc for free in
                        # the fused scalar_tensor_tensor passA.
                        act_drain = j % 4 == 3
                        ps = psmain.tile([128, V], mybir.dt.float32, tag="ps")
                        for jp in range(JP):
                            # [128, 2, m] fp8: one K=256 DoubleRow matmul per
                            # jc pair and PSUM-bank half.
                            last = (jp == JP - 1) and not act_drain
                            lhsT = h[:, 2 * jp:2 * jp + 2, k * PT:k * PT + m]
                            nc.tensor.matmul(
                                ps[:m, 0:NV2], lhsT,
                                wfc_sb[:, 2 * jp:2 * jp + 2, 0:NV2],
                                start=(jp == 0), stop=last,
                                perf_mode=mybir.MatmulPerfMode.DoubleRow)
                            nc.tensor.matmul(
                                ps[:m, NV2:V], lhsT,
                                wfc_sb[:, 2 * jp:2 * jp + 2, NV2:V],
                                start=(jp == 0), stop=last,
                                perf_mode=mybir.MatmulPerfMode.DoubleRow)
                        if act_drain:
                            # fc bias via rank-1 ones x (2^12 b_fc) matmuls
                            nc.tensor.matmul(ps[:m, 0:NV2], ones_sb[0:1, 0:m],
                                             bfc_sb[0:1, 0:NV2], start=False,
                                             stop=True)
                            nc.tensor.matmul(ps[:m, NV2:V], ones_sb[0:1, 0:m],
                                             bfc_sb[0:1, NV2:V], start=False,
                                             stop=True)
                        # passA: scaled+biased logits PSUM -> SBUF bf16. This
                        # is the ONLY op holding the PSUM bank (exp reads ob0,
                        # not PSUM), so the PE never waits on the ACT queue or
                        # the log-softmax chain.
                        ob0 = ob0_pool.tile([128, V], bf16, tag="ob0")
                        if act_drain:
                            nc.scalar.activation(
                                ob0[:m, :], ps[:m, :], AF.Copy,
                                scale=INV_WSCALE)
                        else:
                            nc.vector.scalar_tensor_tensor(
                                ob0[:m, :], ps[:m, :], INV_WSCALE,
                                bfcb_sb[:m, :], Alu.mult, Alu.add)
                        # exp + row-sum accumulation from SBUF bf16 logits
                        # (es itself is discarded; only accum_out matters)
                        es = es_pool.tile([128, V], bf16, tag="es")
                        nc.scalar.activation(
                            es[:m, :], ob0[:m, :], AF.Exp,
                            accum_out=sums[:m, j:j + 1])
                        oct_ob0[j], oct_m[j] = ob0, m
                        oct_r0[j] = blk * ROWS + k * PT
                        if j == OCT - 1:
                            # batched -ln(sums) for 8 tiles, then passB:
                            # out = ob0 + neg_lse (bf16 4x mode) and DMA out.
                            neg_lse = _neg_log_oct(nc, small_pool, sums)
                            last_oct = (blk == NBLK - 1) and (k == NT - 1)
                            for i in range(OCT):
                                obx, mx, r0x = oct_ob0[i], oct_m[i], oct_r0[i]
                                ob = ob_pool.tile([128, V], bf16, tag="ob")
                                nc.vector.tensor_scalar(
                                    ob[:mx, :], obx[:mx, :],
                                    neg_lse[:mx, i:i + 1], None, Alu.add)
                                if last_oct:
                                    # final drain: 4-way split across DMA
                                    # queues so the kernel tail is short
                                    q = (mx + 3) // 4
                                    for c in range(0, mx, q):
                                        ce = min(c + q, mx)
                                        nc.sync.dma_start(
                                            out=out[r0x + c:r0x + ce, :],
                                            in_=ob[c:ce, :])
                                else:
                                    nc.sync.dma_start(
                                        out=out[r0x:r0x + mx, :],
                                        in_=ob[:mx, :])
    nc.finalize()  # runs the Bacc legalization pipeline (wait splitting etc.)
    return nc


def _get_nc():
    if "nc" not in _CACHE:
        _CACHE["nc"] = build_bass()
    return _CACHE["nc"]


def _prep_inputs(encoder_output, decoder_output, W_enc, b_enc, W_dec, b_dec,
                 W_fc, b_fc):
    """Host-side layout prep: transposes, bf16/fp8 casts, bias folding."""
    wenct = np.ascontiguousarray(W_enc.T).astype(BF16)
    wdect = np.ascontiguousarray(W_dec.T).astype(BF16)
    wfct = np.ascontiguousarray(W_fc.T * WSCALE).astype(FP8)
    bjoint = np.ascontiguousarray(
        (b_enc + b_dec).astype(np.float32).reshape(JC, 128).T)
    bfc = (b_fc * WSCALE).reshape(1, V).astype(BF16)
    bfcb = np.ascontiguousarray(
        np.broadcast_to(b_fc.reshape(1, V), (128, V))).astype(BF16)
    in_maps = []
    for b in range(B):
        in_maps.append({
            "enct": np.ascontiguousarray(encoder_output[b].T).astype(BF16),
            "dect": np.ascontiguousarray(decoder_output[b].T).astype(BF16),
            "wenct": wenct,
            "wdect": wdect,
            "wfct": wfct,
            "bjoint": bjoint,
            "bfc": bfc,
            "bfcb": bfcb,
        })
    return in_maps


def kernel(encoder_output, decoder_output, W_enc, b_enc, W_dec, b_dec,
           W_fc, b_fc):
    nc = _get_nc()
    in_maps = _prep_inputs(
        np.asarray(encoder_output), np.asarray(decoder_output),
        np.asarray(W_enc), np.asarray(b_enc), np.asarray(W_dec),
        np.asarray(b_dec), np.asarray(W_fc), np.asarray(b_fc))
    res = run_bass_kernel_spmd(nc, in_maps, core_ids=list(range(B)))
    _CACHE["last_results"] = res
    out = np.empty((B, T, U, V), dtype=np.float32)
    for b in range(B):
        # device rows are (u, t) ordered; reshape + swap to (t, u)
        out[b] = res.results[b]["out"].reshape(U, T, V).transpose(
            1, 0, 2).astype(np.float32)
    return out


# revision 24
# speedup vs baseline: 1.1999x; 1.1999x over previous
"""Trainium2 Bass kernel for an RNN-T style JointNet.

Reference computation (per batch element b):
    enc = enc_out @ W_enc.T + b_enc          # (T, J)
    dec = dec_out @ W_dec.T + b_dec          # (U, J)
    h   = tanh(enc[:,None,:] + dec[None,:,:])  # (T, U, J)
    logits = h @ W_fc.T + b_fc               # (T, U, V)
    out = log_softmax(logits, axis=-1)

Sharding: data-parallel over batch — 8 batch elements, one per NeuronCore.
Device layout: features-on-partitions ("transposed") so the vocab axis of the
logits lands on the free dimension, where the vector/scalar engines can reduce.

v2 changes vs the bf16 baseline (449 us):
  - FC matmul runs in fp8e4 with perf_mode=DoubleRow: h is written by the
    tanh ACT directly as fp8, W_fc is host-quantized fp8 scaled by 2^12
    (dodges e4m3 subnormals; max |W_fc|*4096 = 128 < 240). Each (tile, jc
    pair) is one K=256 DoubleRow matmul — half the matmul count, ~1.44x
    tensor-engine throughput. The 2^12 scale is undone by the free affine
    scale on the Exp and by the fused output op.
  - bf16 everywhere on the DVE: enc_lin/dec_lin/comb are bf16, so the join
    broadcast-add runs in the DVE 4x perf mode.
  - psmain has 4 PSUM buffers (projection pools are scoped out first), so
    the PE never stalls on log-softmax draining.
  - Output is written bf16 (halves the output DMA); host upcasts.
  - fc bias b_fc (scaled 2^12, bf16) still enters via rank-1 ones x b_fc
    accumulating matmuls.
  - The final log_softmax subtract is fused (2^-12 scale + neg_lse add) and
    split between DVE (tensor_scalar) and ACT (Identity w/ bias+scale) to
    balance engine load.
"""

import numpy as np
import ml_dtypes

import concourse.bass as bass
import concourse.mybir as mybir
from concourse import bacc
from concourse.tile import TileContext
from concourse.bass_utils import run_bass_kernel_spmd

BF16 = ml_dtypes.bfloat16
FP8 = ml_dtypes.float8_e4m3

# Problem dims (hardcoded, matches the grading harness inputs)
B, T, U, D, J, V = 8, 200, 50, 512, 1024, 1024
PT = 128          # partition tile (rows per fc matmul tile)
DC = D // 128     # 4 contraction chunks for the projections
JC = J // 128     # 8 contraction chunks for the fc matmul
JP = JC // 2      # 4 DoubleRow k-tile pairs
NV2 = V // 2      # 512: one PSUM bank of fp32
UB = 10           # u values per generation block
NBLK = U // UB    # 5 blocks
ROWS = UB * T     # 2000 rows per block
NT = (ROWS + PT - 1) // PT  # 16 tiles per block (last has 80 rows)

WSCALE = 4096.0   # 2^12: fp8 weight scale (keeps e4m3 out of subnormals)
INV_WSCALE = 1.0 / WSCALE

# log2(1+t) ~= C0 + C1 t + C2 t^2 + C3 t^3 + C4 t^4  (max err 2.1e-4, t in [0,1])
C0, C1, C2, C3, C4 = (
    0.000204257, 1.436097109, -0.669512499, 0.312211590, -0.079149584)
LN2 = 0.6931471805599453

_CACHE = {}


OCT = 8           # log-softmax tiles batched per lse computation


def _neg_log_oct(nc, pool, sums):
    """neg_lse = -ln(sums) for a (128, OCT) fp32 SBUF tile, on the DVE.

    s = 2^e * m with m in [1,2): ln(s) = ln2 * ((e+127) + log2(m) - 127).
    """
    i32, f32 = mybir.dt.int32, mybir.dt.float32
    Alu = mybir.AluOpType
    xi = sums.bitcast(i32)
    e_i = pool.tile([128, OCT], i32, tag="lt_ei")
    nc.vector.tensor_scalar(e_i, xi, 23, None, Alu.logical_shift_right)
    e_f = pool.tile([128, OCT], f32, tag="lt_ef")
    nc.vector.tensor_copy(e_f, e_i)  # int32 -> fp32 value conversion
    m_i = pool.tile([128, OCT], i32, tag="lt_mi")
    nc.vector.tensor_scalar(
        m_i, xi, 0x007FFFFF, 0x3F800000, Alu.bitwise_and, Alu.bitwise_or)
    t = pool.tile([128, OCT], f32, tag="lt_t")
    nc.vector.tensor_scalar(t, m_i.bitcast(f32), 1.0, None, Alu.subtract)
    p = pool.tile([128, OCT], f32, tag="lt_p")
    nc.vector.tensor_scalar(p, t, C4, C3, Alu.mult, Alu.add)
    nc.vector.tensor_mul(p, p, t)
    nc.vector.tensor_scalar(p, p, C2, None, Alu.add)
    nc.vector.tensor_mul(p, p, t)
    nc.vector.tensor_scalar(p, p, C1, None, Alu.add)
    nc.vector.tensor_mul(p, p, t)          # p = P(t) - C0
    nc.vector.tensor_add(p, p, e_f)        # p += (e + 127)
    nl = pool.tile([128, OCT], f32, tag="lt_nl")
    nc.vector.tensor_scalar(nl, p, (C0 - 127.0), -LN2, Alu.add, Alu.mult)
    return nl


def build_bass():
    f32, bf16 = mybir.dt.float32, mybir.dt.bfloat16
    fp8 = mybir.dt.float8e4
    AF = mybir.ActivationFunctionType
    Alu = mybir.AluOpType

    # Bacc (not plain Bass): its compile pipeline legalizes multi-sem waits
    # (1 HW wait slot per instruction) and inserts ACT table loads.
    nc = bacc.Bacc(trn_type="TRN2")
    encT = nc.dram_tensor("enct", [D, T], bf16, kind="ExternalInput")
    decT = nc.dram_tensor("dect", [D, U], bf16, kind="ExternalInput")
    wencT = nc.dram_tensor("wenct", [D, J], bf16, kind="ExternalInput")
    wdecT = nc.dram_tensor("wdect", [D, J], bf16, kind="ExternalInput")
    wfcT = nc.dram_tensor("wfct", [J, V], fp8, kind="ExternalInput")
    bjoint = nc.dram_tensor("bjoint", [128, JC], f32, kind="ExternalInput")
    bfc = nc.dram_tensor("bfc", [1, V], bf16, kind="ExternalInput")
    bfcb = nc.dram_tensor("bfcb", [128, V], bf16, kind="ExternalInput")
    out = nc.dram_tensor("out", [T * U, V], bf16, kind="ExternalOutput")

    with TileContext(nc) as tc:
        with (
            tc.tile_pool(name="const", bufs=1) as const_pool,
            tc.tile_pool(name="comb", bufs=4) as comb_pool,
            tc.tile_pool(name="hbuf", bufs=2) as hbuf_pool,
            tc.tile_pool(name="small", bufs=4) as small_pool,
            tc.tile_pool(name="es", bufs=3) as es_pool,
            # ob0 buffers live for a whole block (16 tiles) awaiting the lse
            tc.tile_pool(name="ob0", bufs=18) as ob0_pool,
            tc.tile_pool(name="ob", bufs=8) as ob_pool,
        ):
            # ---- load constants/weights -------------------------------------
            # enc-path tensors first: the projections only need these.
            encT_sb = const_pool.tile([128, DC, T], bf16)
            nc.sync.dma_start(
                out=encT_sb, in_=encT.rearrange("(c p) t -> p c t", p=128))
            wenc_sb = const_pool.tile([128, DC, J], bf16)
            wenc_r = wencT.rearrange("(c p) j -> p c j", p=128)
            nc.sync.dma_start(out=wenc_sb[:, 0:2, :], in_=wenc_r[:, 0:2, :])
            nc.sync.dma_start(out=wenc_sb[:, 2:4, :], in_=wenc_r[:, 2:4, :])
            decT_sb = const_pool.tile([128, DC, U], bf16)
            nc.sync.dma_start(
                out=decT_sb, in_=decT.rearrange("(c p) u -> p c u", p=128))
            wdec_sb = const_pool.tile([128, DC, J], bf16)
            wdec_r = wdecT.rearrange("(c p) j -> p c j", p=128)
            nc.sync.dma_start(out=wdec_sb[:, 0:2, :], in_=wdec_r[:, 0:2, :])
            nc.sync.dma_start(out=wdec_sb[:, 2:4, :], in_=wdec_r[:, 2:4, :])
            bjoint_sb = const_pool.tile([128, JC], f32)
            nc.sync.dma_start(out=bjoint_sb, in_=bjoint[:, :])
            wfc_sb = const_pool.tile([128, JC, V], fp8)
            wfc_r = wfcT.rearrange("(c p) v -> p c v", p=128)
            nc.sync.dma_start(out=wfc_sb[:, 0:4, :], in_=wfc_r[:, 0:4, :])
            nc.sync.dma_start(out=wfc_sb[:, 4:8, :], in_=wfc_r[:, 4:8, :])
            bfc_sb = const_pool.tile([1, V], bf16)
            nc.sync.dma_start(out=bfc_sb, in_=bfc[:, :])
            # unscaled b_fc broadcast across partitions, for the DVE passA
            bfcb_sb = const_pool.tile([128, V], bf16)
            nc.sync.dma_start(out=bfcb_sb, in_=bfcb[:, :])
            ones_sb = const_pool.tile([1, 128], bf16)
            nc.vector.memset(ones_sb, 1.0)

            # ---- enc/dec projections (feature-on-partition outputs) ---------
            # bf16 outputs: feeds the bf16 4x-mode DVE broadcast-add.
            enc_lin = const_pool.tile([128, JC, T], bf16)
            # f32: tensor_scalar requires an fp32 scalar operand
            dec_lin = const_pool.tile([128, JC, U], f32)
            with (
                # separate 1-buf pools so each projection's first matmul
                # starts on a fresh PSUM slot: matmul instructions have only
                # 2 HW sync-wait slots and the first dec matmul already waits
                # on 2 DMA queues. Scoped: released before psmain opens.
                tc.tile_pool(name="psproj", bufs=1, space="PSUM") as psp,
                tc.tile_pool(name="psdec", bufs=1, space="PSUM") as psd,
            ):
                # interleaved so (enc_lin[jc], dec_lin[jc]) pairs complete
                # early -- block 0's comb/tanh chase right behind
                for jc in range(JC):
                    pe = psp.tile([128, T], f32, tag="proj")
                    for dc in range(DC):
                        nc.tensor.matmul(
                            pe, wenc_sb[:, dc, jc * 128:(jc + 1) * 128],
                            encT_sb[:, dc, :], start=(dc == 0),
                            stop=(dc == DC - 1))
                    nc.scalar.copy(enc_lin[:, jc, :], pe)
                    pd = psd.tile([128, U], f32, tag="dproj")
                    for dc in range(DC):
                        nc.tensor.matmul(
                            pd, wdec_sb[:, dc, jc * 128:(jc + 1) * 128],
                            decT_sb[:, dc, :], start=(dc == 0),
                            stop=(dc == DC - 1))
                    # both biases folded in here: dec_lin += (b_enc + b_dec)
                    nc.scalar.activation(
                        dec_lin[:, jc, :], pd, AF.Identity,
                        bias=bjoint_sb[:, jc:jc + 1], scale=1.0)

            with tc.tile_pool(name="psmain", bufs=4, space="PSUM") as psmain:
                # ---- main loop over u-blocks --------------------------------
                for blk in range(NBLK):
                    h = hbuf_pool.tile([128, JC, ROWS], fp8, tag="h")
                    for jc in range(JC):
                        comb = comb_pool.tile([128, ROWS], bf16, tag="comb")
                        for ul in range(UB):
                            u = blk * UB + ul
                            nc.vector.tensor_scalar(
                                comb[:, ul * T:(ul + 1) * T],
                                enc_lin[:, jc, :],
                                dec_lin[:, jc, u:u + 1], None, Alu.add)
                        nc.scalar.activation(h[:, jc, :], comb, AF.Tanh)

                    oct_ob0 = [None] * OCT
                    oct_m = [0] * OCT
                    oct_r0 = [0] * OCT
                    sums = None
                    for k in range(NT):
                        m = PT if k < NT - 1 else ROWS - PT * (NT - 1)
                        j = k % OCT
                        if j == 0:
                            sums = small_pool.tile(
                                [128, OCT], mybir.dt.float32, tag="sums")
                            nc.vector.memset(sums, 1.0)
                        # Tiles whose passA drains on ACT (engine balance):
                        # ACT can't add the [1,V] bias vector, so these keep
                        # rank-1 bias matmuls. DVE tiles get b_fc for free in
                        # the fused scalar_tensor_tensor passA.
                        act_drain = j % 4 == 3
                        ps = psmain.tile([128, V], mybir.dt.float32, tag="ps")
                        for jp in range(JP):
                            # [128, 2, m] fp8: one K=256 DoubleRow matmul per
                            # jc pair and PSUM-bank half.
                            last = (jp == JP - 1) and not act_drain
                            lhsT = h[:, 2 * jp:2 * jp + 2, k * PT:k * PT + m]
                            nc.tensor.matmul(
                                ps[:m, 0:NV2], lhsT,
                                wfc_sb[:, 2 * jp:2 * jp + 2, 0:NV2],
                                start=(jp == 0), stop=last,
                                perf_mode=mybir.MatmulPerfMode.DoubleRow)
                            nc.tensor.matmul(
                                ps[:m, NV2:V], lhsT,
                                wfc_sb[:, 2 * jp:2 * jp + 2, NV2:V],
                                start=(jp == 0), stop=last,
                                perf_mode=mybir.MatmulPerfMode.DoubleRow)
                        if act_drain:
                            # fc bias via rank-1 ones x (2^12 b_fc) matmuls
                            nc.tensor.matmul(ps[:m, 0:NV2], ones_sb[0:1, 0:m],
                                             bfc_sb[0:1, 0:NV2], start=False,
                                             stop=True)
                            nc.tensor.matmul(ps[:m, NV2:V], ones_sb[0:1, 0:m],
                                             bfc_sb[0:1, NV2:V], start=False,
                                             stop=True)
                        # passA: scaled+biased logits PSUM -> SBUF bf16. This
                        # is the ONLY op holding the PSUM bank (exp reads ob0,
                        # not PSUM), so the PE never waits on the ACT queue or
                        # the log-softmax chain.
                        ob0 = ob0_pool.tile([128, V], bf16, tag="ob0")
                        if act_drain:
                            nc.scalar.activation(
                                ob0[:m, :], ps[:m, :], AF.Copy,
                                scale=INV_WSCALE)
                        else:
                            nc.vector.scalar_tensor_tensor(
                                ob0[:m, :], ps[:m, :], INV_WSCALE,
                                bfcb_sb[:m, :], Alu.mult, Alu.add)
                        # exp + row-sum accumulation from SBUF bf16 logits
                        # (es itself is discarded; only accum_out matters)
                        es = es_pool.tile([128, V], bf16, tag="es")
                        nc.scalar.activation(
                            es[:m, :], ob0[:m, :], AF.Exp,
                            accum_out=sums[:m, j:j + 1])
                        oct_ob0[j], oct_m[j] = ob0, m
                        oct_r0[j] = blk * ROWS + k * PT
                        if j == OCT - 1:
                            # batched -ln(sums) for 8 tiles, then passB:
                            # out = ob0 + neg_lse (bf16 4x mode) and DMA out.
                            neg_lse = _neg_log_oct(nc, small_pool, sums)
                            last_oct = (blk == NBLK - 1) and (k == NT - 1)
                            for i in range(OCT):
                                obx, mx, r0x = oct_ob0[i], oct_m[i], oct_r0[i]
                                ob = ob_pool.tile([128, V], bf16, tag="ob")
                                nc.vector.tensor_scalar(
                                    ob[:mx, :], obx[:mx, :],
                                    neg_lse[:mx, i:i + 1], None, Alu.add)
                                if last_oct:
                                    # final drain: 4-way split across DMA
                                    # queues so the kernel tail is short
                                    q = (mx + 3) // 4
                                    for c in range(0, mx, q):
                                        ce = min(c + q, mx)
                                        nc.sync.dma_start(
                                            out=out[r0x + c:r0x + ce, :],
                                            in_=ob[c:ce, :])
                                else:
                                    nc.sync.dma_start(
                                        out=out[r0x:r0x + mx, :],
                                        in_=ob[:mx, :])
    nc.finalize()  # runs the Bacc legalization pipeline (wait splitting etc.)
    return nc


def _get_nc():
    if "nc" not in _CACHE:
        _CACHE["nc"] = build_bass()
    return _CACHE["nc"]


def _prep_inputs(encoder_output, decoder_output, W_enc, b_enc, W_dec, b_dec,
                 W_fc, b_fc):
    """Host-side layout prep: transposes, bf16/fp8 casts, bias folding."""
    wenct = np.ascontiguousarray(W_enc.T).astype(BF16)
    wdect = np.ascontiguousarray(W_dec.T).astype(BF16)
    wfct = np.ascontiguousarray(W_fc.T * WSCALE).astype(FP8)
    bjoint = np.ascontiguousarray(
        (b_enc + b_dec).astype(np.float32).reshape(JC, 128).T)
    bfc = (b_fc * WSCALE).reshape(1, V).astype(BF16)
    bfcb = np.ascontiguousarray(
        np.broadcast_to(b_fc.reshape(1, V), (128, V))).astype(BF16)
    in_maps = []
    for b in range(B):
        in_maps.append({
            "enct": np.ascontiguousarray(encoder_output[b].T).astype(BF16),
            "dect": np.ascontiguousarray(decoder_output[b].T).astype(BF16),
            "wenct": wenct,
            "wdect": wdect,
            "wfct": wfct,
            "bjoint": bjoint,
            "bfc": bfc,
            "bfcb": bfcb,
        })
    return in_maps


def kernel(encoder_output, decoder_output, W_enc, b_enc, W_dec, b_dec,
           W_fc, b_fc):
    nc = _get_nc()
    in_maps = _prep_inputs(
        np.asarray(encoder_output), np.asarray(decoder_output),
        np.asarray(W_enc), np.asarray(b_enc), np.asarray(W_dec),
        np.asarray(b_dec), np.asarray(W_fc), np.asarray(b_fc))
    res = run_bass_kernel_spmd(nc, in_maps, core_ids=list(range(B)))
    _CACHE["last_results"] = res
    out = np.empty((B, T, U, V), dtype=np.float32)
    for b in range(B):
        # device rows are (u, t) ordered; reshape + swap to (t, u)
        out[b] = res.results[b]["out"].reshape(U, T, V).transpose(
            1, 0, 2).astype(np.float32)
    return out


# revision 25
# speedup vs baseline: 1.2818x; 1.0682x over previous
"""Trainium2 Bass kernel for an RNN-T style JointNet.

Reference computation (per batch element b):
    enc = enc_out @ W_enc.T + b_enc          # (T, J)
    dec = dec_out @ W_dec.T + b_dec          # (U, J)
    h   = tanh(enc[:,None,:] + dec[None,:,:])  # (T, U, J)
    logits = h @ W_fc.T + b_fc               # (T, U, V)
    out = log_softmax(logits, axis=-1)

Sharding: data-parallel over batch — 8 batch elements, one per NeuronCore.
Device layout: features-on-partitions ("transposed") so the vocab axis of the
logits lands on the free dimension, where the vector/scalar engines can reduce.

v2 changes vs the bf16 baseline (449 us):
  - FC matmul runs in fp8e4 with perf_mode=DoubleRow: h is written by the
    tanh ACT directly as fp8, W_fc is host-quantized fp8 scaled by 2^12
    (dodges e4m3 subnormals; max |W_fc|*4096 = 128 < 240). Each (tile, jc
    pair) is one K=256 DoubleRow matmul — half the matmul count, ~1.44x
    tensor-engine throughput. The 2^12 scale is undone by the free affine
    scale on the Exp and by the fused output op.
  - bf16 everywhere on the DVE: enc_lin/dec_lin/comb are bf16, so the join
    broadcast-add runs in the DVE 4x perf mode.
  - psmain has 4 PSUM buffers (projection pools are scoped out first), so
    the PE never stalls on log-softmax draining.
  - Output is written bf16 (halves the output DMA); host upcasts.
  - fc bias b_fc (scaled 2^12, bf16) still enters via rank-1 ones x b_fc
    accumulating matmuls.
  - The final log_softmax subtract is fused (2^-12 scale + neg_lse add) and
    split between DVE (tensor_scalar) and ACT (Identity w/ bias+scale) to
    balance engine load.
"""

import numpy as np
import ml_dtypes

import concourse.bass as bass
import concourse.mybir as mybir
from concourse import bacc
from concourse.tile import TileContext
from concourse.bass_utils import run_bass_kernel_spmd

BF16 = ml_dtypes.bfloat16
FP8 = ml_dtypes.float8_e4m3

# Problem dims (hardcoded, matches the grading harness inputs)
B, T, U, D, J, V = 8, 200, 50, 512, 1024, 1024
PT = 128          # partition tile (rows per fc matmul tile)
DC = D // 128     # 4 contraction chunks for the projections
JC = J // 128     # 8 contraction chunks for the fc matmul
JP = JC // 2      # 4 DoubleRow k-tile pairs
NV2 = V // 2      # 512: one PSUM bank of fp32
UB = 10           # u values per generation block
NBLK = U // UB    # 5 blocks
ROWS = UB * T     # 2000 rows per block
NT = (ROWS + PT - 1) // PT  # 16 tiles per block (last has 80 rows)

WSCALE = 4096.0   # 2^12: fp8 weight scale (keeps e4m3 out of subnormals)
INV_WSCALE = 1.0 / WSCALE

# log2(1+t) ~= C0 + C1 t + C2 t^2 + C3 t^3 + C4 t^4  (max err 2.1e-4, t in [0,1])
C0, C1, C2, C3, C4 = (
    0.000204257, 1.436097109, -0.669512499, 0.312211590, -0.079149584)
LN2 = 0.6931471805599453

_CACHE = {}


OCT = 8           # log-softmax tiles batched per lse computation


def _neg_log_oct(nc, pool, sums):
    """neg_lse = -ln(sums) for a (128, OCT) fp32 SBUF tile, on the DVE.

    s = 2^e * m with m in [1,2): ln(s) = ln2 * ((e+127) + log2(m) - 127).
    """
    i32, f32 = mybir.dt.int32, mybir.dt.float32
    Alu = mybir.AluOpType
    xi = sums.bitcast(i32)
    e_i = pool.tile([128, OCT], i32, tag="lt_ei")
    nc.vector.tensor_scalar(e_i, xi, 23, None, Alu.logical_shift_right)
    e_f = pool.tile([128, OCT], f32, tag="lt_ef")
    nc.vector.tensor_copy(e_f, e_i)  # int32 -> fp32 value conversion
    m_i = pool.tile([128, OCT], i32, tag="lt_mi")
    nc.vector.tensor_scalar(
        m_i, xi, 0x007FFFFF, 0x3F800000, Alu.bitwise_and, Alu.bitwise_or)
    t = pool.tile([128, OCT], f32, tag="lt_t")
    nc.vector.tensor_scalar(t, m_i.bitcast(f32), 1.0, None, Alu.subtract)
    p = pool.tile([128, OCT], f32, tag="lt_p")
    nc.vector.tensor_scalar(p, t, C4, C3, Alu.mult, Alu.add)
    nc.vector.tensor_mul(p, p, t)
    nc.vector.tensor_scalar(p, p, C2, None, Alu.add)
    nc.vector.tensor_mul(p, p, t)
    nc.vector.tensor_scalar(p, p, C1, None, Alu.add)
    nc.vector.tensor_mul(p, p, t)          # p = P(t) - C0
    nc.vector.tensor_add(p, p, e_f)        # p += (e + 127)
    nl = pool.tile([128, OCT], f32, tag="lt_nl")
    nc.vector.tensor_scalar(nl, p, (C0 - 127.0), -LN2, Alu.add, Alu.mult)
    return nl


def build_bass():
    f32, bf16 = mybir.dt.float32, mybir.dt.bfloat16
    fp8 = mybir.dt.float8e4
    AF = mybir.ActivationFunctionType
    Alu = mybir.AluOpType

    # Bacc (not plain Bass): its compile pipeline legalizes multi-sem waits
    # (1 HW wait slot per instruction) and inserts ACT table loads.
    nc = bacc.Bacc(trn_type="TRN2")
    encT = nc.dram_tensor("enct", [D, T], bf16, kind="ExternalInput")
    decT = nc.dram_tensor("dect", [D, U], bf16, kind="ExternalInput")
    wencT = nc.dram_tensor("wenct", [D, J], bf16, kind="ExternalInput")
    wdecT = nc.dram_tensor("wdect", [D, J], bf16, kind="ExternalInput")
    wfcT = nc.dram_tensor("wfct", [J, V], fp8, kind="ExternalInput")
    bjoint = nc.dram_tensor("bjoint", [128, JC], f32, kind="ExternalInput")
    bfc = nc.dram_tensor("bfc", [1, V], bf16, kind="ExternalInput")
    bfcb = nc.dram_tensor("bfcb", [128, V], bf16, kind="ExternalInput")
    out = nc.dram_tensor("out", [T * U, V], bf16, kind="ExternalOutput")

    with TileContext(nc) as tc:
        with (
            tc.tile_pool(name="const", bufs=1) as const_pool,
            tc.tile_pool(name="comb", bufs=4) as comb_pool,
            tc.tile_pool(name="hbuf", bufs=2) as hbuf_pool,
            tc.tile_pool(name="small", bufs=4) as small_pool,
            tc.tile_pool(name="es", bufs=3) as es_pool,
            # ob0 buffers live for a whole block (16 tiles) awaiting the lse
            tc.tile_pool(name="ob0", bufs=18) as ob0_pool,
            tc.tile_pool(name="ob", bufs=8) as ob_pool,
        ):
            # ---- load constants/weights -------------------------------------
            # enc-path tensors first: the projections only need these.
            encT_sb = const_pool.tile([128, DC, T], bf16)
            nc.sync.dma_start(
                out=encT_sb, in_=encT.rearrange("(c p) t -> p c t", p=128))
            wenc_sb = const_pool.tile([128, DC, J], bf16)
            wenc_r = wencT.rearrange("(c p) j -> p c j", p=128)
            nc.sync.dma_start(out=wenc_sb[:, 0:2, :], in_=wenc_r[:, 0:2, :])
            nc.sync.dma_start(out=wenc_sb[:, 2:4, :], in_=wenc_r[:, 2:4, :])
            decT_sb = const_pool.tile([128, DC, U], bf16)
            nc.sync.dma_start(
                out=decT_sb, in_=decT.rearrange("(c p) u -> p c u", p=128))
            wdec_sb = const_pool.tile([128, DC, J], bf16)
            wdec_r = wdecT.rearrange("(c p) j -> p c j", p=128)
            nc.sync.dma_start(out=wdec_sb[:, 0:2, :], in_=wdec_r[:, 0:2, :])
            nc.sync.dma_start(out=wdec_sb[:, 2:4, :], in_=wdec_r[:, 2:4, :])
            bjoint_sb = const_pool.tile([128, JC], f32)
            nc.sync.dma_start(out=bjoint_sb, in_=bjoint[:, :])
            wfc_sb = const_pool.tile([128, JC, V], fp8)
            wfc_r = wfcT.rearrange("(c p) v -> p c v", p=128)
            nc.sync.dma_start(out=wfc_sb[:, 0:4, :], in_=wfc_r[:, 0:4, :])
            nc.sync.dma_start(out=wfc_sb[:, 4:8, :], in_=wfc_r[:, 4:8, :])
            bfc_sb = const_pool.tile([1, V], bf16)
            nc.sync.dma_start(out=bfc_sb, in_=bfc[:, :])
            # unscaled b_fc broadcast across partitions, for the DVE passA
            bfcb_sb = const_pool.tile([128, V], bf16)
            nc.sync.dma_start(out=bfcb_sb, in_=bfcb[:, :])
            ones_sb = const_pool.tile([1, 128], bf16)
            nc.vector.memset(ones_sb, 1.0)

            # ---- enc/dec projections (feature-on-partition outputs) ---------
            # bf16 outputs: feeds the bf16 4x-mode DVE broadcast-add.
            enc_lin = const_pool.tile([128, JC, T], bf16)
            # f32: tensor_scalar requires an fp32 scalar operand
            dec_lin = const_pool.tile([128, JC, U], f32)
            with (
                # separate 1-buf pools so each projection's first matmul
                # starts on a fresh PSUM slot: matmul instructions have only
                # 2 HW sync-wait slots and the first dec matmul already waits
                # on 2 DMA queues. Scoped: released before psmain opens.
                tc.tile_pool(name="psproj", bufs=1, space="PSUM") as psp,
                tc.tile_pool(name="psdec", bufs=1, space="PSUM") as psd,
            ):
                # interleaved so (enc_lin[jc], dec_lin[jc]) pairs complete
                # early -- block 0's comb/tanh chase right behind
                for jc in range(JC):
                    pe = psp.tile([128, T], f32, tag="proj")
                    for dc in range(DC):
                        nc.tensor.matmul(
                            pe, wenc_sb[:, dc, jc * 128:(jc + 1) * 128],
                            encT_sb[:, dc, :], start=(dc == 0),
                            stop=(dc == DC - 1))
                    nc.scalar.copy(enc_lin[:, jc, :], pe)
                    pd = psd.tile([128, U], f32, tag="dproj")
                    for dc in range(DC):
                        nc.tensor.matmul(
                            pd, wdec_sb[:, dc, jc * 128:(jc + 1) * 128],
                            decT_sb[:, dc, :], start=(dc == 0),
                            stop=(dc == DC - 1))
                    # both biases folded in here: dec_lin += (b_enc + b_dec)
                    nc.scalar.activation(
                        dec_lin[:, jc, :], pd, AF.Identity,
                        bias=bjoint_sb[:, jc:jc + 1], scale=1.0)

            with tc.tile_pool(name="psmain", bufs=4, space="PSUM") as psmain:
                # ---- main loop over u-blocks --------------------------------
                for blk in range(NBLK):
                    h = hbuf_pool.tile([128, JC, ROWS], fp8, tag="h")
                    for jc in range(JC):
                        comb = comb_pool.tile([128, ROWS], bf16, tag="comb")
                        for ul in range(UB):
                            u = blk * UB + ul
                            nc.vector.tensor_scalar(
                                comb[:, ul * T:(ul + 1) * T],
                                enc_lin[:, jc, :],
                                dec_lin[:, jc, u:u + 1], None, Alu.add)
                        nc.scalar.activation(h[:, jc, :], comb, AF.Tanh)

                    oct_ob0 = [None] * OCT
                    oct_m = [0] * OCT
                    oct_r0 = [0] * OCT
                    sums = None
                    for k in range(NT):
                        m = PT if k < NT - 1 else ROWS - PT * (NT - 1)
                        j = k % OCT
                        if j == 0:
                            sums = small_pool.tile(
                                [128, OCT], mybir.dt.float32, tag="sums")
                            nc.vector.memset(sums, 1.0)
                        # Tiles whose passA drains on ACT (engine balance):
                        # ACT can't add the [1,V] bias vector, so these keep
                        # rank-1 bias matmuls. DVE tiles get b_fc for free in
                        # the fused scalar_tensor_tensor passA.
                        act_drain = j % 4 == 3
                        ps = psmain.tile([128, V], mybir.dt.float32, tag="ps")
                        for jp in range(JP):
                            # [128, 2, m] fp8: one K=256 DoubleRow matmul per
                            # jc pair and PSUM-bank half.
                            last = (jp == JP - 1) and not act_drain
                            lhsT = h[:, 2 * jp:2 * jp + 2, k * PT:k * PT + m]
                            nc.tensor.matmul(
                                ps[:m, 0:NV2], lhsT,
                                wfc_sb[:, 2 * jp:2 * jp + 2, 0:NV2],
                                start=(jp == 0), stop=last,
                                perf_mode=mybir.MatmulPerfMode.DoubleRow)
                            nc.tensor.matmul(
                                ps[:m, NV2:V], lhsT,
                                wfc_sb[:, 2 * jp:2 * jp + 2, NV2:V],
                                start=(jp == 0), stop=last,
                                perf_mode=mybir.MatmulPerfMode.DoubleRow)
                        if act_drain:
                            # fc bias via rank-1 ones x (2^12 b_fc) matmuls
                            nc.tensor.matmul(ps[:m, 0:NV2], ones_sb[0:1, 0:m],
                                             bfc_sb[0:1, 0:NV2], start=False,
                                             stop=True)
                            nc.tensor.matmul(ps[:m, NV2:V], ones_sb[0:1, 0:m],
                                             bfc_sb[0:1, NV2:V], start=False,
                                             stop=True)
                        # passA: scaled+biased logits PSUM -> SBUF bf16. This
                        # is the ONLY op holding the PSUM bank (exp reads ob0,
                        # not PSUM), so the PE never waits on the ACT queue or
                        # the log-softmax chain.
                        ob0 = ob0_pool.tile([128, V], bf16, tag="ob0")
                        if act_drain:
                            nc.scalar.activation(
                                ob0[:m, :], ps[:m, :], AF.Copy,
                                scale=INV_WSCALE)
                        else:
                            nc.vector.scalar_tensor_tensor(
                                ob0[:m, :], ps[:m, :], INV_WSCALE,
                                bfcb_sb[:m, :], Alu.mult, Alu.add)
                        # exp + row-sum accumulation from SBUF bf16 logits
                        # (es itself is discarded; only accum_out matters)
                        es = es_pool.tile([128, V], bf16, tag="es")
                        nc.scalar.activation(
                            es[:m, :], ob0[:m, :], AF.Exp,
                            accum_out=sums[:m, j:j + 1])
                        oct_ob0[j], oct_m[j] = ob0, m
                        oct_r0[j] = blk * ROWS + k * PT
                        if j == OCT - 1:
                            # batched -ln(sums) for 8 tiles, then passB:
                            # out = ob0 + neg_lse (bf16 4x mode) and DMA out.
                            neg_lse = _neg_log_oct(nc, small_pool, sums)
                            for i in range(OCT):
                                obx, mx, r0x = oct_ob0[i], oct_m[i], oct_r0[i]
                                ob = ob_pool.tile([128, V], bf16, tag="ob")
                                nc.vector.tensor_scalar(
                                    ob[:mx, :], obx[:mx, :],
                                    neg_lse[:mx, i:i + 1], None, Alu.add)
                                nc.sync.dma_start(
                                    out=out[r0x:r0x + mx, :], in_=ob[:mx, :])
    nc.finalize()  # runs the Bacc legalization pipeline (wait splitting etc.)
    return nc


def _get_nc():
    if "nc" not in _CACHE:
        _CACHE["nc"] = build_bass()
    return _CACHE["nc"]


def _prep_inputs(encoder_output, decoder_output, W_enc, b_enc, W_dec, b_dec,
                 W_fc, b_fc):
    """Host-side layout prep: transposes, bf16/fp8 casts, bias folding."""
    wenct = np.ascontiguousarray(W_enc.T).astype(BF16)
    wdect = np.ascontiguousarray(W_dec.T).astype(BF16)
    wfct = np.ascontiguousarray(W_fc.T * WSCALE).astype(FP8)
    bjoint = np.ascontiguousarray(
        (b_enc + b_dec).astype(np.float32).reshape(JC, 128).T)
    bfc = (b_fc * WSCALE).reshape(1, V).astype(BF16)
    bfcb = np.ascontiguousarray(
        np.broadcast_to(b_fc.reshape(1, V), (128, V))).astype(BF16)
    in_maps = []
    for b in range(B):
        in_maps.append({
            "enct": np.ascontiguousarray(encoder_output[b].T).astype(BF16),
            "dect": np.ascontiguousarray(decoder_output[b].T).astype(BF16),
            "wenct": wenct,
            "wdect": wdect,
            "wfct": wfct,
            "bjoint": bjoint,
            "bfc": bfc,
            "bfcb": bfcb,
        })
    return in_maps


def kernel(encoder_output, decoder_output, W_enc, b_enc, W_dec, b_dec,
           W_fc, b_fc):
    nc = _get_nc()
    in_maps = _prep_inputs(
        np.asarray(encoder_output), np.asarray(decoder_output),
        np.asarray(W_enc), np.asarray(b_enc), np.asarray(W_dec),
        np.asarray(b_dec), np.asarray(W_fc), np.asarray(b_fc))
    res = run_bass_kernel_spmd(nc, in_maps, core_ids=list(range(B)))
    _CACHE["last_results"] = res
    out = np.empty((B, T, U, V), dtype=np.float32)
    for b in range(B):
        # device rows are (u, t) ordered; reshape + swap to (t, u)
        out[b] = res.results[b]["out"].reshape(U, T, V).transpose(
            1, 0, 2).astype(np.float32)
    return out
